# revision 1
# baseline (speedup 1.0000x reference)
"""Trainium2 Bass kernel for nn_KmerEmbed: conv1d(one-hot kmer filters) + relu + window-sum.

Computes, for seqs (32,32,30,21), weight (8000,20,3), bias (8000,):
  out[n,m,f] = sum_l relu( sum_{a,j} seqs[n,m,l+j,a(<20)]*weight[f,a,j] + bias[f] )
with l over the 28 valid conv positions; returns (32,32,8000) float32.

Strategy (8 NeuronCores, data-parallel over the 1024 flattened rows, 128 rows/core):
  - im2col on host: X[(j,a)+bias_row, tile, (n4,l28)] -> stationary operand of a
    K=61 matmul streamed against the replicated filter matrix Wt (61,8000) in
    float32r (1 cycle/row at N>=256, ~1e-4 rel precision).
  - conv tiles are packed in pairs into PE row-groups (partitions 0-60 / 64-124)
    so two matmuls stream concurrently.
  - relu(conv) evaluated from PSUM by ScalarE (activation Relu) and VectorE
    (tensor_scalar max) in parallel, written to SBUF as float16.
  - window-sum via a second matmul with 0/1 selection matrices G (112,32) in
    fp16, one PE column-group per 32-row output block; 8 tiles accumulate into
    each column group of a single (128, chunk) PSUM tile, so the final
    PSUM->SBUF copy covers all 128 partitions at once.
  - staging (128,8000) in SBUF, contiguous DMA to DRAM.
"""

import os
import sys

import numpy as np
from numpy.lib.stride_tricks import sliding_window_view

for _p in ("/opt/trn_rl_repo", "/root/.axon_site/_ro/trn_rl_repo"):
    if os.path.isdir(_p) and _p not in sys.path:
        sys.path.insert(0, _p)

import concourse.bacc as bacc
import concourse.mybir as mybir
from concourse.tile import TileContext
from concourse.bass_utils import run_bass_kernel_spmd

# problem sizes (hardcoded per spec)
N_, M_, L_, B_ = 32, 32, 30, 21
A_, K_ = 20, 3
F_ = 8000
NM = N_ * M_              # 1024
CORES = 8
NMC = NM // CORES         # 128 rows per core
LOUT = L_ - K_ + 1        # 28
NMG = 4                   # rows per conv tile
MT = NMG * LOUT           # 112 psum partitions per conv tile
NT = NMC // NMG           # 32 tiles per core
KC = A_ * K_ + 1          # 61 = 60 + bias row
FCH = 500                 # matmul free-dim chunk (one PSUM bank)
BIG = 1000                # relu/copy chunk (two banks)
NBIG = F_ // BIG          # 8

_f32r = mybir.dt.float32r
_f32 = mybir.dt.float32
_f16 = mybir.dt.float16

_cached_nc = None


def _build_program():
    nc = bacc.Bacc("TRN2", target_bir_lowering=False, debug=False,
                   num_devices=CORES)
    xin_d = nc.declare_dram_parameter("xin", [128, NT // 2 * MT], _f16,
                                      isOutput=False)
    wt_d = nc.declare_dram_parameter("wt", [128, F_], _f16, isOutput=False)
    g_d = nc.declare_dram_parameter("g", [MT, 8 * 32], _f16, isOutput=False)
    out_d = nc.declare_dram_parameter("out", [NMC, F_], _f32, isOutput=True)

    relu_fn = mybir.ActivationFunctionType.Relu
    max_op = mybir.AluOpType.max

    with TileContext(nc) as tc:
        with tc.tile_pool(name="const", bufs=1) as cpool, \
             tc.tile_pool(name="rbuf", bufs=18) as rpool, \
             tc.tile_pool(name="stage", bufs=1) as spool, \
             tc.tile_pool(name="pconv", bufs=3, space="PSUM") as pconv, \
             tc.tile_pool(name="psum", bufs=1, space="PSUM") as psump:
            xin_sb = cpool.tile([128, NT // 2 * MT], _f16)
            wt_sb = cpool.tile([128, F_], _f16)
            g_sb = cpool.tile([MT, 8 * 32], _f16)
            stage = spool.tile([NMC, F_], _f32)
            nc.sync.dma_start(out=xin_sb[:], in_=xin_d[:])
            nc.sync.dma_start(out=g_sb[:], in_=g_d[:])
            for i in range(4):
                s = slice(i * (F_ // 4), (i + 1) * (F_ // 4))
                nc.sync.dma_start(out=wt_sb[:, s], in_=wt_d[:, s])

            sum_order = [g0 * 8 + i for i in range(8) for g0 in range(4)]
            first_in_group = set(range(0, NT, 8))

            # pair visit order rotates across the 4 column groups so the
            # window-sum matmuls (which accumulate per column group) can chase
            # the relus instead of bursting at the chunk boundary.
            pair_order = [g + r for r in range(4) for g in [0, 4, 8, 12]]
            for c in range(NBIG):
                rtiles = {}
                ps = psump.tile([128, 1024], _f32)
                started = set()
                n_summed = 0
                for p in pair_order:
                    pc_e = pconv.tile([MT, 1024], _f32, tag="pc")
                    pc_o = pconv.tile([MT, 1024], _f32, tag="pc")
                    for h in range(2):
                        fs = slice(c * BIG + h * FCH, c * BIG + (h + 1) * FCH)
                        hs = slice(h * 512, h * 512 + FCH)
                        nc.tensor.matmul(
                            out=pc_e[:, hs],
                            lhsT=xin_sb[0:KC, p * MT:(p + 1) * MT],
                            rhs=wt_sb[0:KC, fs], start=True, stop=True)
                        nc.tensor.matmul(
                            out=pc_o[:, hs],
                            lhsT=xin_sb[64:64 + KC, p * MT:(p + 1) * MT],
                            rhs=wt_sb[64:64 + KC, fs], start=True, stop=True)
                    r_e = rpool.tile([MT, 1024], _f16, tag="re")
                    r_o = rpool.tile([MT, 1024], _f16, tag="ro")
                    nc.scalar.activation(out=r_e[:, 0:1012], in_=pc_e[:, 0:1012],
                                         func=relu_fn)
                    o_eng = nc.scalar if p == pair_order[-1] else nc.vector
                    if o_eng is nc.scalar:
                        nc.scalar.activation(out=r_o[:, 0:1012],
                                             in_=pc_o[:, 0:1012], func=relu_fn)
                    else:
                        nc.vector.tensor_scalar(out=r_o[:, 0:1012],
                                                in0=pc_o[:, 0:1012],
                                                scalar1=0.0, scalar2=None,
                                                op0=max_op)
                    rtiles[2 * p] = r_e
                    rtiles[2 * p + 1] = r_o
                    # once a full round of 4 pairs (one per column group) is
                    # done, emit their 8 window-sum matmuls (4-way concurrent)
                    if len(rtiles) % 8 == 0:
                        round_pairs = pair_order[len(rtiles) // 2 - 4:
                                                 len(rtiles) // 2]
                        for h in range(2):
                            hs = slice(h * 512, h * 512 + FCH)
                            for dt_ in range(2):
                                for rp in round_pairs:
                                    t = 2 * rp + dt_
                                    grp = t // 8
                                    oi = t % 8
                                    n_summed += 1
                                    is_first = (grp, h) not in started
                                    started.add((grp, h))
                                    nc.tensor.matmul(
                                        out=ps[32 * grp:32 * grp + 32, hs],
                                        lhsT=g_sb[:, 32 * oi:32 * oi + 32],
                                        rhs=rtiles[t][:, hs],
                                        start=is_first,
                                        stop=(n_summed == 2 * NT),
                                        skip_group_check=True,
                                        tile_position=(0, 32 * grp))
                for h in range(2):
                    eng = nc.vector
                    eng.tensor_copy(
                        out=stage[:, c * BIG + h * FCH:c * BIG + (h + 1) * FCH],
                        in_=ps[:, h * 512:h * 512 + FCH])
                if c % 2 == 1:
                    osl = slice((c - 1) * BIG, (c + 1) * BIG)
                    nc.sync.dma_start(out=out_d[:, osl], in_=stage[:, osl])

    nc.compile()
    return nc


def _get_program():
    global _cached_nc
    if _cached_nc is None:
        _cached_nc = _build_program()
    return _cached_nc


def _host_prep(seqs, weight, bias):
    s = np.asarray(seqs, np.float32).reshape(NM, L_, B_)[:, :, :A_]
    sw = sliding_window_view(s, K_, axis=1)          # (NM, 28, 20, 3)
    X = sw.transpose(3, 2, 0, 1).reshape(A_ * K_, NM, LOUT)
    X = np.concatenate([X, np.ones((1, NM, LOUT), np.float32)], axis=0)

    Wt = np.asarray(weight, np.float32).transpose(2, 1, 0).reshape(A_ * K_, F_)
    Wb = np.concatenate([Wt, np.asarray(bias, np.float32)[None, :]], axis=0)
    wt = np.zeros((128, F_), np.float32)
    wt[0:KC] = Wb
    wt[64:64 + KC] = Wb
    wt_f16 = wt.astype(np.float16)

    G = np.zeros((MT, 8 * 32), np.float16)
    for oi in range(8):
        for n in range(NMG):
            G[n * LOUT:(n + 1) * LOUT, 32 * oi + 4 * oi + n] = 1.0

    in_maps = []
    for c in range(CORES):
        Xc = X[:, c * NMC:(c + 1) * NMC, :].reshape(KC, NT, MT)
        xin = np.zeros((128, NT // 2, MT), np.float32)
        xin[0:KC] = Xc[:, 0::2]
        xin[64:64 + KC] = Xc[:, 1::2]
        in_maps.append({
            "xin": np.ascontiguousarray(xin.reshape(128, NT // 2 * MT)).astype(np.float16),
            "wt": wt_f16,
            "g": G,
        })
    return in_maps


def run_bass(seqs, weight, bias, trace=False):
    """Returns (out (32,32,8000) float32, exec_time_ns or None)."""
    nc = _get_program()
    in_maps = _host_prep(seqs, weight, bias)
    res = run_bass_kernel_spmd(nc, in_maps, list(range(CORES)), trace=trace)
    out = np.concatenate([res.results[c]["out"] for c in range(CORES)], axis=0)
    return out.reshape(N_, M_, F_), res.exec_time_ns


def kernel(seqs, weight, bias):
    out, _ = run_bass(seqs, weight, bias, trace=False)
    return out



# revision 2
# speedup vs baseline: 1.8760x; 1.8760x over previous
"""Trainium2 Bass kernel for nn_KmerEmbed: conv1d(one-hot kmer filters) + relu + window-sum.

Computes, for seqs (32,32,30,21), weight (8000,20,3), bias (8000,):
  out[n,m,f] = sum_l relu( s[nm,l,i0] + s[nm,l+1,i1] + s[nm,l+2,i2] - 2 )
where f = i0*400 + i1*20 + i2 (the one-hot kmer filter structure) and
s = seqs[...,:20] flattened to (1024, 30, 20). Returns (32,32,8000) f32.

Strategy (8 cores, data-parallel over the 1024 rows, 128 rows/core,
partitions = the 128 rows n):
  - Pair panels via TensorE: P_l[n, (i1,i2)] = s[n,l+1,i1] + s[n,l+2,i2] - 2
    as a K=41 matmul (one-hot selection + bias row), 28 x 400 cols only
    (~5us instead of a 93us dense conv).
  - Fused build+relu: t_l[n, i0-block] = max(P_l + s[n,l,i0], 0) as a single
    DVE tensor_scalar (op0=add with per-partition scalar, op1=max 0) which
    runs in the 4x DVE perf mode (all-SBUF, f16, packed). A few l-slices go
    to the scalar engine (activation Relu with per-partition bias) to
    balance load.
  - Window-sum over l: identity-stationary matmuls accumulating 28 f16
    tiles into PSUM (the only engine that can sum at 128 elem/cycle).
  - PSUM -> SBUF f32 drain on ScalarE, chunked DMA to DRAM.
"""

import os
import sys

import numpy as np

for _p in ("/opt/trn_rl_repo", "/root/.axon_site/_ro/trn_rl_repo"):
    if os.path.isdir(_p) and _p not in sys.path:
        sys.path.insert(0, _p)

import concourse.bacc as bacc
import concourse.mybir as mybir
from concourse.tile import TileContext
from concourse.bass_utils import run_bass_kernel_spmd

# problem sizes (hardcoded per spec)
N_, M_, L_, B_ = 32, 32, 30, 21
A_, K_ = 20, 3
F_ = 8000
NM = N_ * M_              # 1024
CORES = 8
NMC = NM // CORES         # 128 rows per core
LOUT = L_ - K_ + 1        # 28 conv positions
KP = 2 * A_ + 1           # 41 contraction rows for the pair panel matmul
NI2 = A_ * A_             # 400 = one (i1,i2) block / one i0 f-block
CH = 2 * NI2              # 800 = f macro-chunk (2 i0 blocks)
NCH = F_ // CH            # 10 macro-chunks

_f32 = mybir.dt.float32
_f16 = mybir.dt.float16

# l values whose build blocks run on the scalar engine (load balance):
SCALAR_LS = frozenset((3, 10, 17, 24))

_cached_nc = None


def _build_program():
    nc = bacc.Bacc("TRN2", target_bir_lowering=False, debug=False,
                   num_devices=CORES)
    xl_d = nc.declare_dram_parameter("xl", [KP, LOUT * NMC], _f16,
                                     isOutput=False)
    at_d = nc.declare_dram_parameter("at", [NMC, LOUT * A_], _f32,
                                     isOutput=False)
    w2_d = nc.declare_dram_parameter("w2", [KP, NI2], _f16, isOutput=False)
    id_d = nc.declare_dram_parameter("idm", [NMC, NMC], _f16, isOutput=False)
    out_d = nc.declare_dram_parameter("out", [NMC, F_], _f32, isOutput=True)

    add_op = mybir.AluOpType.add
    max_op = mybir.AluOpType.max
    copy_fn = mybir.ActivationFunctionType.Copy
    relu_fn = mybir.ActivationFunctionType.Relu

    with TileContext(nc) as tc:
        with tc.tile_pool(name="const", bufs=1) as cpool, \
             tc.tile_pool(name="trelu", bufs=2) as tpool, \
             tc.tile_pool(name="stage", bufs=2) as spool, \
             tc.tile_pool(name="psp", bufs=2, space="PSUM") as psp, \
             tc.tile_pool(name="pss", bufs=4, space="PSUM") as pss:
            xl_sb = cpool.tile([KP, LOUT * NMC], _f16)
            at_sb = cpool.tile([NMC, LOUT * A_], _f32)
            w2_sb = cpool.tile([KP, NI2], _f16)
            id_sb = cpool.tile([NMC, NMC], _f16)
            p_sb = cpool.tile([NMC, LOUT * NI2], _f16)

            nc.sync.dma_start(out=xl_sb[:], in_=xl_d[:])
            nc.sync.dma_start(out=at_sb[:], in_=at_d[:])
            nc.sync.dma_start(out=w2_sb[:], in_=w2_d[:])
            nc.sync.dma_start(out=id_sb[:], in_=id_d[:])

            # phase 1: pair panels P_l[n, (i1,i2)] = b + c - 2
            for l in range(LOUT):
                pp = psp.tile([NMC, NI2], _f32, tag="pp")
                nc.tensor.matmul(out=pp[:],
                                 lhsT=xl_sb[:, l * NMC:(l + 1) * NMC],
                                 rhs=w2_sb[:], start=True, stop=True)
                nc.scalar.activation(out=p_sb[:, l * NI2:(l + 1) * NI2],
                                     in_=pp[:], func=copy_fn)

            # phase 2/3 pipelined per macro-chunk:
            #   build t_relu (DVE 4x tensor_scalar / ScalarE activation),
            #   then accumulate over l via identity matmuls into PSUM.
            for c in range(NCH):
                tr = tpool.tile([NMC, LOUT * CH], _f16, tag="tr")
                for l in range(LOUT):
                    src = p_sb[:, l * NI2:(l + 1) * NI2]
                    for h in range(2):
                        i0 = 2 * c + h
                        dst = tr[:, l * CH + h * NI2: l * CH + (h + 1) * NI2]
                        sc = at_sb[:, l * A_ + i0: l * A_ + i0 + 1]
                        if l in SCALAR_LS:
                            nc.scalar.activation(out=dst, in_=src,
                                                 func=relu_fn, bias=sc,
                                                 scale=1.0)
                        else:
                            nc.vector.tensor_scalar(out=dst, in0=src,
                                                    scalar1=sc, scalar2=0.0,
                                                    op0=add_op, op1=max_op)
                st = spool.tile([NMC, CH], _f32, tag="st")
                for h in range(2):
                    ps = pss.tile([NMC, NI2], _f32, tag="ps")
                    for l in range(LOUT):
                        nc.tensor.matmul(
                            out=ps[:], lhsT=id_sb[:],
                            rhs=tr[:, l * CH + h * NI2: l * CH + (h + 1) * NI2],
                            start=(l == 0), stop=(l == LOUT - 1))
                    nc.scalar.activation(out=st[:, h * NI2:(h + 1) * NI2],
                                         in_=ps[:], func=copy_fn)
                nc.sync.dma_start(out=out_d[:, c * CH:(c + 1) * CH], in_=st[:])

    nc.compile()
    return nc


def _get_program():
    global _cached_nc
    if _cached_nc is None:
        _cached_nc = _build_program()
    return _cached_nc


def _host_prep(seqs, weight, bias):
    s = np.asarray(seqs, np.float32).reshape(NM, L_, B_)[:, :, :A_]

    e = np.eye(A_, dtype=np.float32)
    w2 = np.zeros((KP, NI2), np.float32)
    w2[0:A_] = np.repeat(e, A_, axis=1)       # [a == i1]
    w2[A_:2 * A_] = np.tile(e, (1, A_))       # [a == i2]
    w2[2 * A_, :] = -2.0                      # bias row
    w2_f16 = w2.astype(np.float16)
    idm = np.eye(NMC, dtype=np.float16)

    in_maps = []
    for c in range(CORES):
        sc_ = s[c * NMC:(c + 1) * NMC]        # (128, 30, 20)
        xl = np.ones((KP, LOUT, NMC), np.float32)
        xl[0:A_] = sc_[:, 1:1 + LOUT, :].transpose(2, 1, 0)
        xl[A_:2 * A_] = sc_[:, 2:2 + LOUT, :].transpose(2, 1, 0)
        at = sc_[:, :LOUT, :].reshape(NMC, LOUT * A_)
        in_maps.append({
            "xl": np.ascontiguousarray(xl.reshape(KP, LOUT * NMC)).astype(np.float16),
            "at": np.ascontiguousarray(at, dtype=np.float32),
            "w2": w2_f16,
            "idm": idm,
        })
    return in_maps


def run_bass(seqs, weight, bias, trace=False):
    """Returns (out (32,32,8000) float32, exec_time_ns or None)."""
    nc = _get_program()
    in_maps = _host_prep(seqs, weight, bias)
    res = run_bass_kernel_spmd(nc, in_maps, list(range(CORES)), trace=trace)
    out = np.concatenate([res.results[c]["out"] for c in range(CORES)], axis=0)
    return out.reshape(N_, M_, F_), res.exec_time_ns


def kernel(seqs, weight, bias):
    out, _ = run_bass(seqs, weight, bias, trace=False)
    return out
